# revision 66
# baseline (speedup 1.0000x reference)
"""Trainium2 Bass kernel for nn_AttractorLayerUnnormed.

Reference computation (full inputs x [4,256,96,128], b_prev [4,64,48,64],
w1 [128,256], b1 [128], w2 [16,128], b2 [16]):
  hid = relu(w1 @ x + b1)                    (1x1 conv)
  A   = softplus(w2 @ hid + b2)              [n, 16, 96, 128]
  b_c = bilinear_resize(b_prev, 96, 128)     (align_corners) [n, 64, 96, 128]
  out = b_c + sum_a (A_a - b_c) * exp(-300 (A_a - b_c)^2)

Sharding: 8 cores = (sample n) x (h-half); each core owns 48 rows x 128 cols
= 6144 positions, processed as 12 chunks of F=512.

Default variant "v9" (sim 93.1us/core vs 186us for the v6 baseline; HW
K-loop slope ~75-105us/core, v6 measures 178us back-to-back). Key facts discovered on the way:
  - PE matmuls cost 4 cycles/row in fp32 but 1 cycle/row in float32r
    (relaxed, tf32-ish) when the moving free dim >= 256, and 1 in bf16.
    All big matmuls here are fp32r or bf16; BIR requires every PRODUCER
    of an fp32r matmul operand to emit fp32r (DMA/ACT/DVE outs).
  - error budget (tol 2e-2): resize/b path must stay >= fp32r precision
    (bf16 b alone costs 1.1e-2 because alpha=300 makes the gaussian
    razor thin); x/w1/hid/w2 in bf16 cost ~3-4e-3 each; e/term bf16
    are negligible. Measured end-to-end: 4.6e-3.
  - LoadActFuncSet costs 1.3us; the Bacc scheduler interleaves Exp/Ln,
    so the module-level activation-table patch strips Exp/Ln from all
    sets except natural_log_exp_and_others (act_func_set_id is the
    POSITION in act_info.json - never reorder the dict).
  - HWDGE issues one DMA per ~625ns: xs is host-transposed to [128,2,S]
    bf16 so one DMA loads 2 chunks x both K-halves.

Phase 1 (~25us, DMA/latency-bound): per chunk one xs DMA (2 chunks),
mm1 (bf16) -> relu+bias (DVE tensor_scalar, per-partition bias AP) ->
mm2 (bf16) -> Exp (ACT); resize matmuls (fp32r, deferred 6 chunks so PE
stays off the mm critical chain; bsel DMA'd in halves mid-stream) -> psum->ab copy (DVE); Ln per 4
chunks into ab A-rows (no global softplus barrier).

Phase 3 (~60us, ACT/DVE balanced at ~58us busy each, >95% occupancy):
ab packed [80 rows = 64 b + 16 A] (32-aligned partition bases, no dead
rows, no memset); per chunk 8 dx matmuls (fp32r K=80, j-pairs in
[128,2F] PSUM tiles) -> gaussian e = Derivative_Erf(sqrt(300)dx) in ONE
ACT pass -> term = dx*e (DVE TT, bf16 out) -> delta accumulated on PE
(sselb9 bf16 weights pre-scaled by sqrt(pi)/2) PLUS b via an id80 fp32r
matmul into the same psum_d -> out copy on ACT -> DMA. Chunks are
software-pipelined: chunk c's reduction is emitted after chunk c+1's
dx/gauss/mult, which removes ~0.5us/chunk of cross-engine stalls.
"""

import numpy as np

import concourse.bacc as bacc
import concourse.tile as tile
from concourse import mybir
from concourse.bass_utils import run_bass_kernel_spmd

# Prefer the combined exp+ln table set so the scheduler's interleaving of
# Exp and Ln ops doesn't trigger a LoadActFuncSet (~1.3us) per alternation.
# act_func_set_id is the POSITION in act_info.json, so the dict order must be
# preserved: instead of reordering, strip Exp/Ln from the competing sets so
# the chooser can only satisfy them with natural_log_exp_and_others.
_orig_gat = bacc.get_activation_tables


def _gat_pref_combined(arch):
    t = _orig_gat(arch)
    pref = "natural_log_exp_and_others"
    if pref not in t:
        return t
    AF = mybir.ActivationFunctionType
    return {
        k: (v if k == pref else v - {AF.Exp, AF.Ln})
        for k, v in t.items()
    }


bacc.get_activation_tables = _gat_pref_combined

ALPHA = 300.0
N_CORES = 8
S = 48 * 128  # positions per core
NCHUNK = 12
F = 512  # positions per chunk
SQRT_A = float(np.sqrt(ALPHA))

# which j-iterations compute sq on DVE (rest on ACT) - load balance knob
DVE_SQ_JS = (0, 2, 5)

_CACHE = {}


def _f32(x):
    return np.ascontiguousarray(x, dtype=np.float32)


def _host_prep(inputs):
    x = np.asarray(inputs["x"], dtype=np.float32)
    b_prev = np.asarray(inputs["b_prev"], dtype=np.float32)
    w1 = np.asarray(inputs["w1"], dtype=np.float32)
    b1 = np.asarray(inputs["b1"], dtype=np.float32)
    w2 = np.asarray(inputs["w2"], dtype=np.float32)
    b2 = np.asarray(inputs["b2"], dtype=np.float32)

    H, W, h_in, w_in = 96, 128, 48, 64

    ys = np.linspace(0.0, h_in - 1.0, H)
    y0 = np.floor(ys).astype(np.int64)
    wy = (ys - y0).astype(np.float32)
    xs_ = np.linspace(0.0, w_in - 1.0, W)
    x0 = np.floor(xs_).astype(np.int64)
    x1 = np.minimum(x0 + 1, w_in - 1)
    wx = (xs_ - x0).astype(np.float32)

    CxT = np.zeros((w_in, W), dtype=np.float32)
    CxT[x0, np.arange(W)] += 1.0 - wx
    CxT[x1, np.arange(W)] += wx

    per_core = []
    for core in range(N_CORES):
        n, half = core // 2, core % 2
        h0 = half * 48
        y0l = y0[h0 : h0 + 48]
        wyl = wy[h0 : h0 + 48]

        xs_c = _f32(x[n, :, h0 : h0 + 48, :].reshape(2, 128, S))

        bp_t = b_prev[n].transpose(2, 1, 0)  # [l, k, bin]
        Bsel = np.empty((2, 64, 48, 64), dtype=np.float32)
        for j in range(2):
            wj = (1.0 - wyl) if j == 0 else wyl  # fold row-interp weights in
            Bsel[j] = bp_t[:, np.clip(y0l + j, 0, 47), :] * wj[None, :, None]
        Bsel = _f32(Bsel.reshape(128, 48, 64))

        per_core.append({"xs": xs_c, "bsel": Bsel})

    m = np.arange(128)
    consts = {
        "w1t": _f32(w1.T.reshape(2, 128, 128)),
        "w2t": _f32(w2.T),  # [128, 16]
        "b1": _f32(b1.reshape(128, 1)),
        "b2": _f32(np.concatenate([b2, np.zeros(112, np.float32)]).reshape(128, 1)),
        "asel": _f32(np.arange(16)[:, None] == (m[None, :] % 16)),  # [16, 128]
        "nball": None,  # filled below
        "sseljb": None,  # filled below
        "nbselj": _f32(
            -np.stack(
                [
                    (np.arange(64)[:, None] == (8 * j + m[None, :] // 16)).astype(
                        np.float32
                    )
                    for j in range(8)
                ],
                axis=1,
            )
        ),  # [64, 8, 128]
        "sselj": _f32(
            np.stack(
                [
                    ((8 * j + m[:, None] // 16) == np.arange(64)[None, :])
                    for j in range(8)
                ],
                axis=1,
            )
        ),  # [128, 8, 64]
        "ones": np.ones((128, 1), dtype=np.float32),
        "cxt2": _f32(np.concatenate([CxT, CxT], axis=0)),  # [128, 128]
    }
    asel = consts["asel"]
    nbselj = consts["nbselj"]  # [64, 8, 128]
    nball = np.zeros((128, 8, 128), dtype=np.float32)
    for j in range(8):
        nball[:16, j, :] = asel
        nball[64:, j, :] = nbselj[:, j, :]
    consts["nball"] = _f32(nball)
    import ml_dtypes

    bf = ml_dtypes.bfloat16
    consts["sseljb"] = consts["sselj"].astype(bf)

    # --- v9 extras ---
    # ab packed [80]: rows 0:64 = b bins, rows 64:80 = A (32-aligned bases)
    nball80 = np.zeros((80, 8, 128), dtype=np.float32)
    for j in range(8):
        nball80[:64, j, :] = nbselj[:, j, :]
        nball80[64:, j, :] = asel
    consts["nball80"] = _f32(nball80)
    # sum weights pre-scaled by sqrt(pi)/2 (undoes Derivative_Erf's 2/sqrt(pi))
    consts["sselb9"] = (consts["sselj"] * 0.8862269254527580).astype(bf)
    consts["id80"] = _f32(
        np.concatenate([np.eye(64, dtype=np.float32), np.zeros((16, 64), np.float32)])
    )
    consts["w1tb"] = consts["w1t"].astype(bf)
    consts["bias3"] = _f32(
        np.concatenate([consts["b1"], consts["b2"], consts["ones"]], axis=1)
    )  # [128, 3]
    consts["w2tb"] = consts["w2t"].astype(bf)
    consts["cxt2b"] = consts["cxt2"].astype(bf)
    for pc in per_core:
        hi = pc["bsel"].astype(bf)
        lo = (pc["bsel"] - hi.astype(np.float32)).astype(bf)
        pc["bselhi"] = hi
        pc["bsello"] = lo
    consts["w1tb2"] = np.ascontiguousarray(
        consts["w1t"].transpose(1, 0, 2)
    ).astype(bf)  # [128, 2, 128]
    for pc in per_core:
        pc["xsb2"] = np.ascontiguousarray(
            pc["xs"].transpose(1, 0, 2)
        ).astype(bf)  # [128, 2, S]
    return per_core, consts


def _build_v9(outer_iters=1, relu_dve=True, copy_dve=True, badd_pe=True):
    """v9: two phases.

    Phase 1 (DMA/ACT/DVE-light): per chunk mm1 (bf16) -> relu+bias (DVE
    tensor_scalar) -> mm2 (bf16) -> Exp (ACT); resize matmuls (bf16) ->
    copy psum->ab (DVE); Ln per 4 chunks (removes the global softplus
    barrier). Phase 3 (balanced ~58us/engine): dx matmuls (fp32r, K=80
    packed ab, no memset), gaussian via Derivative_Erf (ACT), term mult
    (DVE), sum + b-add both PE-accumulated into psum_d (id64 weight,
    sqrt(pi)/2 folded into sselb9), out copy on ACT.
    """
    nc = bacc.Bacc(None, target_bir_lowering=False)
    dt = mybir.dt.float32
    dtr = mybir.dt.float32r
    bf = mybir.dt.bfloat16
    AF = mybir.ActivationFunctionType
    OP = mybir.AluOpType

    xsb2 = nc.dram_tensor("xsb2", [128, 2, S], bf, kind="ExternalInput")
    bselr = nc.dram_tensor("bsel", [128, 48, 64], dtr, kind="ExternalInput")
    cxt2r = nc.dram_tensor("cxt2", [128, 128], dtr, kind="ExternalInput")
    w1tb2 = nc.dram_tensor("w1tb2", [128, 2, 128], bf, kind="ExternalInput")
    w2tb = nc.dram_tensor("w2tb", [128, 16], bf, kind="ExternalInput")
    bias3 = nc.dram_tensor("bias3", [128, 3], dt, kind="ExternalInput")
    sselb9 = nc.dram_tensor("sselb9", [128, 8, 64], bf, kind="ExternalInput")
    nball80 = nc.dram_tensor("nball80", [80, 8, 128], dtr, kind="ExternalInput")
    id80 = nc.dram_tensor("id80", [80, 64], dtr, kind="ExternalInput")
    out = nc.dram_tensor("out", [64, 48, 128], dt, kind="ExternalOutput")

    with tile.TileContext(nc) as tc:
        with (
            tc.tile_pool(name="singles", bufs=1) as singles,
            tc.tile_pool(name="xin", bufs=3) as xin,
            tc.tile_pool(name="work", bufs=3) as work,
            tc.tile_pool(name="jwork", bufs=3) as jwork,
            tc.tile_pool(name="terms", bufs=10) as terms_pool,
        ):
            # small consts first; big/late-needed tensors are DMA'd at the
            # point they are needed (bsel after chunk-0 xs, nball/sselb/id80
            # after the mm loop) to keep the issue queue clear for xs.
            w1t_sb = singles.tile([128, 2, 128], bf)
            w2t_sb = singles.tile([128, 16], bf)
            bias3_sb = singles.tile([128, 3], dt)
            cxt2_sb = singles.tile([128, 128], dtr)
            id80_sb = singles.tile([80, 64], dtr)
            bsel_sb = singles.tile([128, 48, 64], dtr)
            sselb_sb = singles.tile([128, 8, 64], bf)
            nball_sb = singles.tile([80, 8, 128], dtr)
            ab_all = singles.tile([80, NCHUNK * F], dtr)
            ez_all = singles.tile([16, NCHUNK * F], dt)

            import contextlib

            loop_cm = (
                tc.For_i(0, outer_iters, 1)
                if outer_iters > 1
                else contextlib.nullcontext()
            )
            with loop_cm:
                # pb9 spans both phases (resize c8-11 is spliced into early
                # phase 3). PSUM peak: phase1 ph2+pz2+pb2=6; phase3
                # pdx(2x2)+pd2+pb2=8.
                with contextlib.ExitStack() as pb_stack:
                    pb9 = pb_stack.enter_context(
                        tc.tile_pool(name="pb9", bufs=3, space="PSUM")
                    )

                    def resize(c, on_dve):
                        sl = slice(c * F, (c + 1) * F)
                        psum_b = pb9.tile([64, 4, 128], dt, name="psum_b")
                        for yl in range(4):
                            nc.tensor.matmul(
                                psum_b[:, yl, :],
                                bsel_sb[:, 4 * c + yl, :],
                                cxt2_sb[:, :],
                                start=True,
                                stop=True,
                            )
                        bflat = psum_b[:, :, :].rearrange("p a b -> p (a b)")
                        if on_dve:
                            nc.vector.tensor_copy(ab_all[0:64, sl], bflat)
                        else:
                            nc.scalar.activation(ab_all[0:64, sl], bflat, AF.Copy)

                    # ---- phase 1: mm + softplus + resize c0-7 ----
                    with (
                        tc.tile_pool(name="ph9", bufs=2, space="PSUM") as ph9,
                        tc.tile_pool(name="pz9", bufs=2, space="PSUM") as pz9,
                    ):
                        xts = {}
                        for c in range(NCHUNK):
                            sl = slice(c * F, (c + 1) * F)
                            if c % 2 == 0:
                                sl2 = slice(c * F, (c + 2) * F)
                                xt2 = xin.tile([128, 2, 2 * F], bf, tag="xt")
                                nc.sync.dma_start(out=xt2, in_=xsb2[:, :, sl2])
                                xts[c // 2] = xt2
                            xt2 = xts[c // 2]
                            xt = xt2[:, :, (c % 2) * F : (c % 2 + 1) * F]
                            if c == 0:
                                nc.sync.dma_start(out=w1t_sb, in_=w1tb2[:, :, :])
                                nc.sync.dma_start(out=w2t_sb, in_=w2tb[:, :])
                                nc.sync.dma_start(out=bias3_sb, in_=bias3[:, :])
                                nc.sync.dma_start(out=cxt2_sb, in_=cxt2r[:, :])
                            psum_h = ph9.tile([128, F], dt)
                            nc.tensor.matmul(
                                psum_h, w1t_sb[:, 0, :], xt[:, 0, :],
                                start=True, stop=False,
                            )
                            nc.tensor.matmul(
                                psum_h, w1t_sb[:, 1, :], xt[:, 1, :],
                                start=False, stop=True,
                            )
                            hid = work.tile([128, F], bf, tag="hid")
                            if relu_dve:
                                nc.vector.tensor_scalar(
                                    hid, psum_h, bias3_sb[:, 0:1], 0.0,
                                    op0=OP.add, op1=OP.max,
                                )
                            else:
                                nc.scalar.activation(
                                    hid, psum_h, AF.Relu, bias=bias3_sb[:, 0:1]
                                )
                            psum_z = pz9.tile([16, F], dt)
                            nc.tensor.matmul(psum_z, w2t_sb, hid, start=True, stop=True)
                            nc.scalar.activation(
                                ez_all[:, sl], psum_z, AF.Exp, bias=bias3_sb[:16, 1:2]
                            )
                            if c >= 2:
                                cc = c - 2
                                resize(cc, on_dve=(cc >= 6))
                            if c == 0:
                                nc.sync.dma_start(
                                    out=bsel_sb[:, 0:24, :], in_=bselr[:, 0:24, :]
                                )
                            if c == 4:
                                nc.sync.dma_start(
                                    out=bsel_sb[:, 24:48, :], in_=bselr[:, 24:48, :]
                                )
                            if c == 11:
                                nc.sync.dma_start(out=sselb_sb, in_=sselb9[:, :, :])
                                nc.sync.dma_start(out=nball_sb, in_=nball80[:, :, :])
                                nc.sync.dma_start(out=id80_sb, in_=id80[:, :])
                            # Ln rides the combined exp+ln table set: no
                            # switch. The last group is a single chunk so the
                            # table-switch gate (last Ln) lands ~1.3us sooner.
                            ln_groups = {3: 4, 7: 4, 10: 3, 11: 1}
                            if c in ln_groups:
                                n_ch = ln_groups[c]
                                sl4 = slice((c - n_ch + 1) * F, (c + 1) * F)
                                nc.scalar.activation(
                                    ab_all[64:80, sl4],
                                    ez_all[:, sl4],
                                    AF.Ln,
                                    bias=bias3_sb[:16, 2:3],
                                )
                    resize(NCHUNK - 2, on_dve=True)
                    resize(NCHUNK - 1, on_dve=True)
                    pb_stack.close()
                    # ---- phase 3: attractor, software-pipelined ----
                    with (
                        tc.tile_pool(name="pdx9", bufs=3, space="PSUM") as pdx9,
                        tc.tile_pool(name="pd9", bufs=2, space="PSUM") as pd9,
                    ):
                        state = {}

                        def front(c):
                            sl = slice(c * F, (c + 1) * F)
                            dx_pairs = []
                            terms = []
                            for p in range(4):
                                pdx2 = pdx9.tile([128, 2, F], dt, tag="dx2")
                                for i in range(2):
                                    nc.tensor.matmul(
                                        pdx2[:, i, :],
                                        nball_sb[:, 2 * p + i, :],
                                        ab_all[:, sl],
                                        start=True,
                                        stop=True,
                                    )
                                flat = pdx2[:, :, :].rearrange("p a b -> p (a b)")
                                e_t = jwork.tile([128, 2 * F], bf, tag="et")
                                nc.scalar.activation(
                                    e_t, flat, AF.Derivative_Erf, scale=SQRT_A
                                )
                                term = terms_pool.tile([128, 2, F], bf, tag="tm")
                                nc.vector.tensor_tensor(
                                    term[:, :, :].rearrange("p a b -> p (a b)"),
                                    flat,
                                    e_t,
                                    op=OP.mult,
                                )
                                terms.append(term)
                            state[c] = terms

                        def back(c):
                            sl = slice(c * F, (c + 1) * F)
                            terms = state.pop(c)
                            psum_d = pd9.tile([64, F], dt, name="psum_d")
                            if badd_pe:
                                nc.tensor.matmul(
                                    psum_d, id80_sb, ab_all[:, sl],
                                    start=True, stop=False,
                                )
                            for j in range(8):
                                nc.tensor.matmul(
                                    psum_d,
                                    sselb_sb[:, j, :],
                                    terms[j // 2][:, j % 2, :],
                                    start=(not badd_pe) and (j == 0),
                                    stop=(j == 7),
                                )
                            out_t = work.tile([64, F], dt, tag="ot")
                            nc.scalar.activation(out_t, psum_d, AF.Copy)
                            nc.sync.dma_start(
                                out=out[:, 4 * c : 4 * c + 4, :],
                                in_=out_t[:, :].rearrange("p (a b) -> p a b", a=4),
                            )

                        import os
                        if os.environ.get("V9_NOPIPE"):
                            for c in range(NCHUNK):
                                front(c)
                                back(c)
                        else:
                            for c in range(NCHUNK):
                                front(c)
                                if c >= 1:
                                    back(c - 1)
                            back(NCHUNK - 1)
    nc.compile()
    return nc


def _build_bass(variant="v9", outer_iters=1):
    if variant == "v9":
        return _build_v9(outer_iters=outer_iters)
    nc = bacc.Bacc(None, target_bir_lowering=False)
    dt = mybir.dt.float32
    # v8: tensors feeding matmuls are float32r (1 cyc/row on PE vs 4 for fp32)
    dtr = mybir.dt.float32r if variant == "v8" else dt
    AF = mybir.ActivationFunctionType
    OP = mybir.AluOpType

    xs = nc.dram_tensor("xs", [2, 128, S], dtr, kind="ExternalInput")
    bsel = nc.dram_tensor("bsel", [128, 48, 64], dtr, kind="ExternalInput")
    cxt2 = nc.dram_tensor("cxt2", [128, 128], dtr, kind="ExternalInput")
    w1t = nc.dram_tensor("w1t", [2, 128, 128], dtr, kind="ExternalInput")
    w2t = nc.dram_tensor("w2t", [128, 16], dtr, kind="ExternalInput")
    b1 = nc.dram_tensor("b1", [128, 1], dt, kind="ExternalInput")
    b2 = nc.dram_tensor("b2", [128, 1], dt, kind="ExternalInput")
    asel = nc.dram_tensor("asel", [16, 128], dt, kind="ExternalInput")
    nbselj = nc.dram_tensor("nbselj", [64, 8, 128], dt, kind="ExternalInput")
    sselj = nc.dram_tensor("sselj", [128, 8, 64], dt, kind="ExternalInput")
    sseljb = nc.dram_tensor("sseljb", [128, 8, 64], mybir.dt.bfloat16, kind="ExternalInput")
    nball = nc.dram_tensor("nball", [128, 8, 128], dtr, kind="ExternalInput")
    ones = nc.dram_tensor("ones", [128, 1], dt, kind="ExternalInput")
    out = nc.dram_tensor("out", [64, 48, 128], dt, kind="ExternalOutput")

    with tile.TileContext(nc) as tc:
        with (
            tc.tile_pool(name="singles", bufs=1) as singles,
            tc.tile_pool(name="xin", bufs=3) as xin,
            tc.tile_pool(name="work", bufs=2) as work,
            tc.tile_pool(name="small", bufs=2) as small,
            tc.tile_pool(name="jwork", bufs=3) as jwork,
            tc.tile_pool(name="terms", bufs=10) as terms_pool,
            tc.tile_pool(name="ph", bufs=1, space="PSUM") as ph,
            tc.tile_pool(name="pz", bufs=1, space="PSUM") as pz,
            tc.tile_pool(
                name="pb", bufs=1, space="PSUM"
            ) as pb,
            tc.tile_pool(
                name="pdx",
                bufs=(4 if variant in ("pipe", "allsqdve") else 2),
                space="PSUM",
            ) as pdx,
            tc.tile_pool(
                name="pd",
                bufs=(1 if variant in ("pipe", "allsqdve", "v3") else 2),
                space="PSUM",
            ) as pd,
        ):
            # resident weights / constants
            w1t_sb = singles.tile([128, 2, 128], dtr)
            nc.sync.dma_start(out=w1t_sb[:, 0, :], in_=w1t[0])
            nc.sync.dma_start(out=w1t_sb[:, 1, :], in_=w1t[1])
            w2t_sb = singles.tile([128, 16], dtr)
            nc.sync.dma_start(out=w2t_sb, in_=w2t[:, :])
            b1_sb = singles.tile([128, 1], dt)
            nc.sync.dma_start(out=b1_sb, in_=b1[:, :])
            b2_sb = singles.tile([128, 1], dt)
            nc.sync.dma_start(out=b2_sb, in_=b2[:, :])
            ones_sb = singles.tile([128, 1], dt)
            nc.sync.dma_start(out=ones_sb, in_=ones[:, :])
            stacked = variant in ("v2", "v3", "v4", "v5", "v6", "v8")
            if not stacked:
                asel_sb = singles.tile([16, 128], dt)
                nc.sync.dma_start(out=asel_sb, in_=asel[:, :])
                nbsel_sb = singles.tile([64, 8, 128], dt)
                nc.sync.dma_start(out=nbsel_sb, in_=nbselj[:, :, :])
                ssel_sb = singles.tile([128, 8, 64], dt)
                nc.sync.dma_start(out=ssel_sb, in_=sselj[:, :, :])
            else:
                sselb_sb = singles.tile([128, 8, 64], mybir.dt.bfloat16)
                nc.sync.dma_start(out=sselb_sb, in_=sseljb[:, :, :])
                nball_sb = singles.tile([128, 8, 128], dtr)
                nc.sync.dma_start(out=nball_sb, in_=nball[:, :, :])
                ab_all = singles.tile([128, NCHUNK * F], dtr)
                nc.vector.memset(ab_all[0:64, :].bitcast(dt), 0.0)
                ez_all = singles.tile([16, NCHUNK * F], dt)
            bsel_sb = singles.tile([128, 48, 64], dtr)
            nc.sync.dma_start(out=bsel_sb, in_=bsel[:, :, :])
            cxt2_sb = singles.tile([128, 128], dtr)
            nc.sync.dma_start(out=cxt2_sb, in_=cxt2[:, :])

            import contextlib

            loop_cm = (
                tc.For_i(0, outer_iters, 1)
                if outer_iters > 1
                else contextlib.nullcontext()
            )
            r_ = (
                (lambda ap: ap.bitcast(mybir.dt.float32r))
                if variant == "v8"
                else (lambda ap: ap)
            )
            with loop_cm:
              if variant in ("v4", "v5", "v6", "v8"):
                with tc.tile_pool(name="phv4", bufs=2, space="PSUM") as ph4, tc.tile_pool(
                    name="pzv4", bufs=2, space="PSUM"
                ) as pz4:
                    for c in range(NCHUNK):
                        sl = slice(c * F, (c + 1) * F)
                        x0t = xin.tile([128, F], dtr, tag="xt")
                        x1t = xin.tile([128, F], dtr, tag="xt")
                        nc.sync.dma_start(out=x0t, in_=xs[0, :, sl])
                        nc.sync.dma_start(out=x1t, in_=xs[1, :, sl])
                        psum_h = ph4.tile([128, F], dt)
                        nc.tensor.matmul(
                            psum_h, r_(w1t_sb[:, 0, :]), r_(x0t), start=True, stop=False
                        )
                        nc.tensor.matmul(
                            psum_h, r_(w1t_sb[:, 1, :]), r_(x1t), start=False, stop=True
                        )
                        hid = work.tile([128, F], dtr, tag="hid")
                        nc.scalar.activation(hid, psum_h, AF.Relu, bias=b1_sb[:, 0:1])
                        psum_z = pz4.tile([16, F], dt)
                        nc.tensor.matmul(psum_z, r_(w2t_sb), r_(hid), start=True, stop=True)
                        nc.scalar.activation(
                            ez_all[:, sl], psum_z, AF.Exp, bias=b2_sb[:16, 0:1]
                        )
                        if variant == "v5" and c % 2 == 1:
                            sl2 = slice((c - 1) * F, (c + 1) * F)
                            nc.scalar.activation(
                                ab_all[:16, sl2],
                                ez_all[:, sl2],
                                AF.Ln,
                                bias=ones_sb[:16, 0:1],
                            )
                    if variant != "v5":
                        nc.scalar.activation(
                            ab_all[:16, :], ez_all, AF.Ln, bias=ones_sb[:16, 0:1]
                        )
                # resize phase: scoped pb pool
                with tc.tile_pool(name="pbv4", bufs=2, space="PSUM") as pb4:
                    for c in range(NCHUNK):
                        sl = slice(c * F, (c + 1) * F)
                        psum_b = pb4.tile([64, 4, 128], dt)
                        for yl in range(4):
                            y = 4 * c + yl
                            nc.tensor.matmul(
                                psum_b[:, yl, :],
                                r_(bsel_sb[:, y, :]),
                                r_(cxt2_sb[:, :]),
                                start=True,
                                stop=True,
                            )
                        nc.scalar.activation(
                            ab_all[64:, sl],
                            psum_b[:, :, :].rearrange("p a b -> p (a b)"),
                            AF.Copy,
                        )
                with tc.tile_pool(name="pdxv4", bufs=3, space="PSUM") as pdx4, tc.tile_pool(
                    name="pdv4", bufs=2, space="PSUM"
                ) as pd4:
                    for c in range(NCHUNK):
                        sl = slice(c * F, (c + 1) * F)
                        psum_d = pd4.tile([64, F], dt)
                        dx_pairs = []
                        for p in range(4):
                            pdx2 = pdx4.tile([128, 2, F], dt, tag="dx2")
                            for i in range(2):
                                nc.tensor.matmul(
                                    pdx2[:, i, :],
                                    r_(nball_sb[:, 2 * p + i, :]),
                                    r_(ab_all[:, sl]),
                                    start=True,
                                    stop=True,
                                )
                            dx_pairs.append(pdx2)
                        terms = []
                        for p in range(4):
                            pdx2 = dx_pairs[p]
                            flat = pdx2[:, :, :].rearrange("p a b -> p (a b)")
                            e_t = jwork.tile([128, 2 * F], dt, tag="et")
                            term = terms_pool.tile(
                                [128, 2, F], mybir.dt.bfloat16, tag="tm"
                            )
                            if variant in ("v6", "v8"):
                                # erf'(x) = (2/sqrt(pi)) exp(-x^2): one ACT op
                                # computes the gaussian; the 2/sqrt(pi) is
                                # divided back out in the final add.
                                nc.scalar.activation(
                                    e_t, flat, AF.Derivative_Erf, scale=SQRT_A
                                )
                            else:
                                sq = jwork.tile([128, 2 * F], dt, tag="sq")
                                nc.scalar.activation(
                                    sq, flat, AF.Square, scale=SQRT_A
                                )
                                nc.scalar.activation(e_t, sq, AF.Exp, scale=-1.0)
                            nc.vector.tensor_tensor(
                                term[:, :, :].rearrange("p a b -> p (a b)"),
                                flat,
                                e_t,
                                op=OP.mult,
                            )
                            terms.append(term)
                        for j in range(8):
                            nc.tensor.matmul(
                                psum_d,
                                sselb_sb[:, j, :],
                                terms[j // 2][:, j % 2, :],
                                start=(j == 0),
                                stop=(j == 7),
                            )
                        out_t = work.tile([64, F], dt, tag="ot")
                        if variant in ("v6", "v8"):
                            nc.vector.scalar_tensor_tensor(
                                out_t,
                                psum_d,
                                0.8862269254527580,
                                ab_all[64:, sl],
                                op0=OP.mult,
                                op1=OP.add,
                            )
                        else:
                            nc.vector.tensor_add(out_t, psum_d, ab_all[64:, sl])
                        nc.sync.dma_start(
                            out=out[:, 4 * c : 4 * c + 4, :],
                            in_=out_t[:, :].rearrange("p (a b) -> p a b", a=4),
                        )
              elif variant == "v3":
                # ---- resize first (independent of x): fills ab_all[16:80] ----
                for c in range(NCHUNK):
                    sl = slice(c * F, (c + 1) * F)
                    psum_b = pb.tile([64, 4, 128], dt)
                    for yl in range(4):
                        y = 4 * c + yl
                        nc.tensor.matmul(
                            psum_b[:, yl, :],
                            bsel_sb[:, y, :],
                            cxt2_sb[:, :],
                            start=True,
                            stop=True,
                        )
                    nc.scalar.activation(
                        ab_all[64:, sl],
                        psum_b[:, :, :].rearrange("p a b -> p (a b)"),
                        AF.Copy,
                    )
                # ---- phase 1: mm1+relu+mm2+exp; one Ln ----
                for c in range(NCHUNK):
                    sl = slice(c * F, (c + 1) * F)
                    x0t = xin.tile([128, F], dt, tag="xt")
                    x1t = xin.tile([128, F], dt, tag="xt")
                    nc.sync.dma_start(out=x0t, in_=xs[0, :, sl])
                    nc.sync.dma_start(out=x1t, in_=xs[1, :, sl])
                    psum_h = ph.tile([128, F], dt)
                    nc.tensor.matmul(
                        psum_h, w1t_sb[:, 0, :], x0t, start=True, stop=False
                    )
                    nc.tensor.matmul(
                        psum_h, w1t_sb[:, 1, :], x1t, start=False, stop=True
                    )
                    hid = work.tile([128, F], dt, tag="hid")
                    nc.scalar.activation(hid, psum_h, AF.Relu, bias=b1_sb[:, 0:1])
                    psum_z = pz.tile([16, F], dt)
                    nc.tensor.matmul(psum_z, w2t_sb, hid, start=True, stop=True)
                    nc.scalar.activation(
                        ez_all[:, sl], psum_z, AF.Exp, bias=b2_sb[:16, 0:1]
                    )
                nc.scalar.activation(
                    ab_all[:16, :], ez_all, AF.Ln, bias=ones_sb[:16, 0:1]
                )
                # ---- phase 2: attractor, j-pairs batched ----
                for c in range(NCHUNK):
                    sl = slice(c * F, (c + 1) * F)
                    psum_d = pd.tile([64, F], dt)
                    dx_pairs = []
                    for p in range(4):
                        pdx2 = pdx.tile([128, 2, F], dt, tag="dx2")
                        for i in range(2):
                            nc.tensor.matmul(
                                pdx2[:, i, :],
                                nball_sb[:, 2 * p + i, :],
                                ab_all[:, sl],
                                start=True,
                                stop=True,
                            )
                        dx_pairs.append(pdx2)
                    terms = []
                    for p in range(4):
                        pdx2 = dx_pairs[p]
                        flat = pdx2[:, :, :].rearrange("p a b -> p (a b)")
                        sq = jwork.tile([128, 2 * F], dt, tag="sq")
                        e_t = jwork.tile([128, 2 * F], dt, tag="et")
                        term = terms_pool.tile(
                            [128, 2, F], mybir.dt.bfloat16, tag="tm"
                        )
                        nc.scalar.activation(sq, flat, AF.Square, scale=SQRT_A)
                        nc.scalar.activation(e_t, sq, AF.Exp, scale=-1.0)
                        nc.vector.tensor_tensor(
                            term[:, :, :].rearrange("p a b -> p (a b)"),
                            flat,
                            e_t,
                            op=OP.mult,
                        )
                        terms.append(term)
                    for j in range(8):
                        nc.tensor.matmul(
                            psum_d,
                            sselb_sb[:, j, :],
                            terms[j // 2][:, j % 2, :],
                            start=(j == 0),
                            stop=(j == 7),
                        )
                    out_t = work.tile([64, F], dt, tag="ot")
                    nc.vector.tensor_add(out_t, psum_d, ab_all[64:, sl])
                    nc.sync.dma_start(
                        out=out[:, 4 * c : 4 * c + 4, :],
                        in_=out_t[:, :].rearrange("p (a b) -> p a b", a=4),
                    )
              elif variant == "v2":
                # ---- phase 1: mm1+relu+mm2+exp for all chunks; one Ln ----
                for c in range(NCHUNK):
                    sl = slice(c * F, (c + 1) * F)
                    x0t = xin.tile([128, F], dt, tag="xt")
                    x1t = xin.tile([128, F], dt, tag="xt")
                    nc.sync.dma_start(out=x0t, in_=xs[0, :, sl])
                    nc.sync.dma_start(out=x1t, in_=xs[1, :, sl])
                    psum_h = ph.tile([128, F], dt)
                    nc.tensor.matmul(
                        psum_h, w1t_sb[:, 0, :], x0t, start=True, stop=False
                    )
                    nc.tensor.matmul(
                        psum_h, w1t_sb[:, 1, :], x1t, start=False, stop=True
                    )
                    hid = work.tile([128, F], dt, tag="hid")
                    nc.scalar.activation(hid, psum_h, AF.Relu, bias=b1_sb[:, 0:1])
                    psum_z = pz.tile([16, F], dt)
                    nc.tensor.matmul(psum_z, w2t_sb, hid, start=True, stop=True)
                    nc.scalar.activation(
                        ez_all[:, sl], psum_z, AF.Exp, bias=b2_sb[:16, 0:1]
                    )
                # softplus tail: A = Ln(ez + 1), into the top 16 rows of ab_all
                nc.scalar.activation(
                    ab_all[:16, :], ez_all, AF.Ln, bias=ones_sb[:16, 0:1]
                )
                # ---- phase 2: resize + attractor ----
                for c in range(NCHUNK):
                    sl = slice(c * F, (c + 1) * F)
                    psum_b = pb.tile([64, 4, 128], dt)
                    for yl in range(4):
                        y = 4 * c + yl
                        nc.tensor.matmul(
                            psum_b[:, yl, :],
                            bsel_sb[:, y, :],
                            cxt2_sb[:, :],
                            start=True,
                            stop=True,
                        )
                    nc.scalar.activation(
                        ab_all[64:, sl],
                        psum_b[:, :, :].rearrange("p a b -> p (a b)"),
                        AF.Copy,
                    )
                    psum_d = pd.tile([64, F], dt)
                    dxs_tiles = []
                    for j in range(8):
                        psum_dx = pdx.tile([128, F], dt, tag="dx")
                        nc.tensor.matmul(
                            psum_dx,
                            nball_sb[:, j, :],
                            ab_all[:, sl],
                            start=True,
                            stop=True,
                        )
                        dxs_tiles.append(psum_dx)
                    terms = []
                    for j in range(8):
                        psum_dx = dxs_tiles[j]
                        sq = jwork.tile([128, F], dt, tag="sq")
                        e_t = jwork.tile([128, F], dt, tag="et")
                        term = terms_pool.tile(
                            [128, F], mybir.dt.bfloat16, tag="tm"
                        )
                        nc.scalar.activation(sq, psum_dx, AF.Square, scale=SQRT_A)
                        nc.scalar.activation(e_t, sq, AF.Exp, scale=-1.0)
                        nc.vector.tensor_tensor(term, psum_dx, e_t, op=OP.mult)
                        terms.append(term)
                    for j in range(8):
                        nc.tensor.matmul(
                            psum_d,
                            sselb_sb[:, j, :],
                            terms[j],
                            start=(j == 0),
                            stop=(j == 7),
                        )
                    out_t = work.tile([64, F], dt, tag="ot")
                    nc.vector.tensor_add(out_t, psum_d, ab_all[64:, sl])
                    nc.sync.dma_start(
                        out=out[:, 4 * c : 4 * c + 4, :],
                        in_=out_t[:, :].rearrange("p (a b) -> p a b", a=4),
                    )
              else:
                for c in range(NCHUNK):
                  sl = slice(c * F, (c + 1) * F)
                  # ---- mm1 + relu ----
                  x0t = xin.tile([128, F], dt, tag="xt")
                  x1t = xin.tile([128, F], dt, tag="xt")
                  nc.sync.dma_start(out=x0t, in_=xs[0, :, sl])
                  nc.sync.dma_start(out=x1t, in_=xs[1, :, sl])
                  psum_h = ph.tile([128, F], dt)
                  nc.tensor.matmul(psum_h, w1t_sb[:, 0, :], x0t, start=True, stop=False)
                  nc.tensor.matmul(psum_h, w1t_sb[:, 1, :], x1t, start=False, stop=True)
                  hid = work.tile([128, F], dt, tag="hid")
                  nc.scalar.activation(hid, psum_h, AF.Relu, bias=b1_sb[:, 0:1])

                  # ---- mm2 + softplus (Exp then Ln(1+x)) ----
                  psum_z = pz.tile([16, F], dt)
                  nc.tensor.matmul(psum_z, w2t_sb, hid, start=True, stop=True)
                  ez = small.tile([16, F], dt, tag="ez")
                  nc.scalar.activation(ez, psum_z, AF.Exp, bias=b2_sb[:16, 0:1])
                  a_t = small.tile([16, F], dt, tag="at")
                  nc.scalar.activation(a_t, ez, AF.Ln, bias=ones_sb[:16, 0:1])

                  # ---- bilinear resize: 4 output rows per chunk ----
                  psum_b = pb.tile([64, 4, 128], dt)
                  for yl in range(4):
                      y = 4 * c + yl
                      nc.tensor.matmul(
                          psum_b[:, yl, :],
                          bsel_sb[:, y, :],
                          cxt2_sb[:, :],
                          start=True,
                          stop=True,
                      )
                  b_tile = work.tile([64, F], dt, tag="bt")
                  nc.scalar.activation(
                      b_tile, psum_b[:, :, :].rearrange("p a b -> p (a b)"), AF.Copy
                  )

                  # ---- attractor loop ----
                  psum_d = pd.tile([64, F], dt)
                  if variant == "nojl":
                      nc.tensor.matmul(
                          psum_d, ssel_sb[:, 0, :], hid, start=True, stop=True
                      )
                  else:
                      dve_js = () if variant == "allact" else (
                          tuple(range(8)) if variant == "allsqdve" else DVE_SQ_JS
                      )
                      # emit dx matmuls first (wave-limited by pdx bufs), then the
                      # elementwise chains, then the accumulating sum matmuls -
                      # keeps PE fed ahead of the ACT/DVE latency chain.
                      dxs_tiles = []
                      for j in range(8):
                          psum_dx = pdx.tile([128, F], dt, tag="dx")
                          nc.tensor.matmul(psum_dx, asel_sb, a_t, start=True, stop=False)
                          nc.tensor.matmul(
                              psum_dx, nbsel_sb[:, j, :], b_tile, start=False, stop=True
                          )
                          dxs_tiles.append(psum_dx)
                      terms = []
                      for j in range(8):
                          psum_dx = dxs_tiles[j]
                          sq = jwork.tile([128, F], dt, tag="sq")
                          term = terms_pool.tile([128, F], dt, tag="tm")
                          e_t = jwork.tile([128, F], dt, tag="et")
                          if j in dve_js:
                              dxs = jwork.tile([128, F], dt, tag="dxs")
                              nc.vector.tensor_copy(dxs, psum_dx)
                              nc.vector.scalar_tensor_tensor(
                                  sq, dxs, ALPHA, dxs, op0=OP.mult, op1=OP.mult
                              )
                              nc.scalar.activation(e_t, sq, AF.Exp, scale=-1.0)
                              nc.vector.tensor_tensor(term, dxs, e_t, op=OP.mult)
                          else:
                              nc.scalar.activation(sq, psum_dx, AF.Square, scale=SQRT_A)
                              nc.scalar.activation(e_t, sq, AF.Exp, scale=-1.0)
                              nc.vector.tensor_tensor(term, psum_dx, e_t, op=OP.mult)
                          terms.append(term)
                      for j in range(8):
                          nc.tensor.matmul(
                              psum_d,
                              ssel_sb[:, j, :],
                              terms[j],
                              start=(j == 0),
                              stop=(j == 7),
                          )

                  # ---- final add + store ----
                  out_t = work.tile([64, F], dt, tag="ot")
                  nc.vector.tensor_add(out_t, psum_d, b_tile)
                  nc.sync.dma_start(
                      out=out[:, 4 * c : 4 * c + 4, :],
                      in_=out_t[:, :].rearrange("p (a b) -> p a b", a=4),
                  )

    nc.compile()
    return nc


def _get_nc():
    if "nc" not in _CACHE:
        _CACHE["nc"] = _build_bass()
    return _CACHE["nc"]


def kernel(**inputs):
    nc = _get_nc()
    per_core, consts = _host_prep(inputs)
    # only ship tensors the module declares (host prep also builds inputs for
    # older kernel variants)
    declared = set()
    for alloc in nc.m.functions[0].allocations:
        if isinstance(alloc, mybir.MemoryLocationSet) and alloc.kind == "ExternalInput":
            declared.add(alloc.memorylocations[0].name)
    in_maps = [
        {k: v for k, v in dict(consts, **pc).items() if k in declared}
        for pc in per_core
    ]
    res = run_bass_kernel_spmd(nc, in_maps, core_ids=list(range(N_CORES)))
    out = np.zeros((4, 64, 96, 128), dtype=np.float32)
    for core in range(N_CORES):
        n, half = core // 2, core % 2
        out[n, :, half * 48 : half * 48 + 48, :] = res.results[core]["out"]
    return out

